# revision 38
# baseline (speedup 1.0000x reference)
"""NT-Xent (SimCLR) contrastive loss on 8 Trainium2 NeuronCores — v3 (fp8).

Symmetric half-band design as v2: exp(sim) is symmetric, so each global row i
only computes columns at circular distance d = j-i mod 2N in [1, 4096]; every
unordered pair lands on exactly one core except d == 4096 (the positive pair),
which lands on both and is corrected on the host.

v3 changes vs v2:
- z is normalized on the HOST (f32) and shipped as fp8e4m3 in a DoubleRow
  plane layout [128, 2, COLS] (partition p holds dims p and p+128). This
  deletes the whole on-device normalization pipeline (squares, norm matmuls,
  rsqrt chain, broadcast DMAs) that caused multi-us dependency bubbles, and
  cuts input DMA 4x.
- All matmuls run fp8 DoubleRow (0.5 cyc/row, K=256 in one pass): the sim
  matmul needs one instruction per 512 output cols, and the exp'd band tiles
  of two adjacent row-tiles are column-summed in one paired matmul.
- exp runs on ACT from [128,1536] PSUM tiles (3 per row-tile: 1536+1536+1152)
  with accum_out giving f32 row sums for free; e is written back as fp8 only
  for the colsum matmul. ACT is the bottleneck engine (~39 us busy/core).
- Column sums go per-pair straight from PSUM to DRAM (rows 2p,2p+1 of the
  [8, BAND] output), no DVE evacuation.

Host assembles den from f32 row sums + fp8-rounded col sums, subtracts the
double-counted positive exp, and takes mean(log(D) - 2*pos) with pos in f32.
"""

import sys

for _p in ("/opt/trn_rl_repo",):
    if _p not in sys.path:
        sys.path.insert(0, _p)

import ml_dtypes
import numpy as np

import concourse.bass as bass
import concourse.tile as tile
from concourse import bacc, mybir
from concourse.bass_utils import run_bass_kernel_spmd

F32 = mybir.dt.float32
F8 = mybir.dt.float8e4
AF = mybir.ActivationFunctionType
DR = mybir.MatmulPerfMode.DoubleRow
NP_F8 = ml_dtypes.float8_e4m3

N_CORES = 8
N = 4096
D = 256
TWO_N = 2 * N            # 8192 rows of sim
ROWS = TWO_N // N_CORES  # 1024 rows per core
COLS = 5120              # rotated columns staged per core
BAND = 4224              # band columns per 128-row tile (4096 + 128 wedge)
NEG_MASK = -128.0        # fp8-exact; exp(2*(sim-128)) underflows to 0
SEGS = ((0, 1536), (1536, 1536), (3072, 1152))  # band segments per row-tile

_CACHE = {}
LAST_RESULTS = None


def _plane3(base: bass.AP, off: int, plane_stride: int, w: int) -> bass.AP:
    """[128, 2, w] DoubleRow view of a plane-major [128, 2*S] sbuf tile."""
    return bass.AP(
        tensor=base.tensor,
        offset=base.offset + off,
        ap=[list(base.ap[0]), [plane_stride, 2], [1, w]],
    )


def _build_nc() -> bass.Bass:
    nc = bacc.Bacc("TRN2", num_devices=N_CORES)

    z_d = nc.dram_tensor("z8", [128, 2 * COLS], F8, kind="ExternalInput")
    # mask-matmul constants packed in one tensor: [mlo | mup | ident]
    mc_d = nc.dram_tensor("mconst", [128, 384], F8, kind="ExternalInput")
    rows_d = nc.dram_tensor("rows", [128, 8], F32, kind="ExternalOutput")
    cols_d = nc.dram_tensor("cols", [8, BAND], F32, kind="ExternalOutput")

    with tile.TileContext(nc) as tc:
        with (
            tc.tile_pool(name="big", bufs=1) as big,
            tc.tile_pool(name="ep", bufs=2) as ep,
            tc.tile_pool(name="cg", bufs=2) as cg,
            tc.tile_pool(name="small", bufs=1) as small,
            tc.tile_pool(name="ps", bufs=2, space="PSUM") as ps,
            tc.tile_pool(name="cs", bufs=2, space="PSUM") as cs,
        ):
            z8 = big.tile([128, 2 * COLS], F8, tag="z8")
            # Input DMAs: first halves of BOTH planes first (rt0 needs both),
            # spread across engine queues so the ~600ns issue costs overlap
            H = COLS // 2
            mconst = small.tile([128, 384], F8, tag="mconst")
            mask_lo = mconst[:, 0:128]
            mask_up = mconst[:, 128:256]
            ident = mconst[:, 256:384]
            # 3 chunks per plane: [0:1664] covers rt0's first segment, so
            # compute starts after ~1/3 of the load; later chunks land before
            # the row-tiles that need them
            for a, b in ((0, 1664), (1664, 3392), (3392, COLS)):
                nc.sync.dma_start(out=z8[:, a:b], in_=z_d.ap()[:, a:b])
                nc.scalar.dma_start(
                    out=z8[:, COLS + a : COLS + b],
                    in_=z_d.ap()[:, COLS + a : COLS + b],
                )
                if a == 0:
                    nc.gpsimd.dma_start(out=mconst[:, :], in_=mc_d.ap()[:, :])

            # pair selector [128, 2, 128]: plane 0 -> row 0 (even row-tile),
            # plane 1 -> row 1 (odd row-tile); every pair's colsums land at
            # psum partitions 0:2 (engine partition-base must be 0/32/64/96).
            # Full 128-col stationary: narrower ones fail the LDW ISA check.
            self_f = small.tile([128, 256], F32, tag="selftmp")
            nc.vector.memset(self_f[:, :], 0.0)
            nc.vector.memset(self_f[:, 0:1], 1.0)
            nc.vector.memset(self_f[:, 129:130], 1.0)
            sel = small.tile([128, 256], F8, tag="sel")
            nc.vector.tensor_copy(sel[:, :], self_f[:, :])

            rowsums = small.tile([128, 24], F32, tag="rsum")
            zbase = z8[:, :]

            rsum8 = small.tile([128, 8], F32, tag="rs8")

            # colsum emission is deferred by TWO segments: the PE runs a
            # segment ahead of ACT, so a colsum placed one segment later
            # still head-of-line blocks the pq fill behind its exps; two
            # segments later it slots into genuinely idle PE time.
            state = {"cstage": None, "pending": []}

            def emit_colsum(p, et_pair, off, w, last):
                if off == 0:
                    cstage_t = cg.tile([2, BAND], F32, tag="cstage")
                    state["cstage"] = cstage_t
                cstage = state["cstage"]
                eb = et_pair[:, :]
                for ci, sub0 in enumerate(range(off, off + w, 512)):
                    sw = min(512, off + w - sub0)
                    cst = cs.tile([128, 512], F32, tag="cs")
                    nc.tensor.matmul(
                        cst[0:128, 0:sw],
                        _plane3(sel[:, :], 0, 128, 128),
                        _plane3(eb, sub0, BAND, sw),
                        start=True,
                        stop=True,
                        perf_mode=DR,
                    )
                    if last and ci % 2 == 1:
                        # ACT is done with exps by now; split the tail copies
                        # across ACT and DVE
                        nc.scalar.copy(
                            cstage[0:2, sub0 : sub0 + sw], cst[0:2, 0:sw]
                        )
                    else:
                        nc.vector.tensor_copy(
                            cstage[0:2, sub0 : sub0 + sw], cst[0:2, 0:sw]
                        )
                nc.sync.dma_start(
                    out=cols_d.ap()[2 * p : 2 * p + 2, off : off + w],
                    in_=cstage[0:2, off : off + w],
                )

            et = None
            for rt in range(8):
                if rt == 7:
                    # rowsums only lack rt7's segments; queue the reduce and
                    # its DMA (scalar queue) now so they overlap the final
                    # colsum chain instead of trailing it
                    nc.vector.tensor_reduce(
                        rsum8[:, 0:7],
                        rowsums[:, 0:21].rearrange("p (a b) -> p a b", b=3),
                        axis=mybir.AxisListType.X,
                        op=mybir.AluOpType.add,
                    )
                if rt % 2 == 0:
                    et = ep.tile([128, 2 * BAND], F8, tag="e")
                pl = rt % 2
                ebase = et[:, :]
                for si, (off, w) in enumerate(SEGS):
                    pq = ps.tile([128, 1536], F32, tag="pq")
                    lhsT = _plane3(zbase, rt * 128, COLS, 128)
                    for sub0 in range(0, w, 512):
                        sw = min(512, w - sub0)
                        # the masked sub-tiles keep their accumulation group
                        # open for the mask matmul below
                        masked = (si == 0 and sub0 == 0) or (
                            si == 2 and sub0 == 1024
                        )
                        nc.tensor.matmul(
                            pq[:, sub0 : sub0 + sw],
                            lhsT,
                            _plane3(zbase, rt * 128 + off + sub0, COLS, sw),
                            start=True,
                            stop=not masked,
                            perf_mode=DR,
                        )
                    if si == 0:
                        # first band tile: mask distance <= 0 (diag+lower):
                        # pq[p,t] += NEG*1[t<=p] as a matmul (lhsT[k,m] =
                        # NEG*1[k<=m], rhs=I), keeping the mask on the PE so
                        # no other engine gates the exp
                        nc.tensor.matmul(
                            pq[:, 0:128],
                            mask_lo,
                            ident,
                            start=False,
                            stop=True,
                            skip_group_check=True,
                        )
                    if si == 2:
                        # wedge (distance ~4096): mask d > 4096
                        nc.tensor.matmul(
                            pq[:, 1024:1152],
                            mask_up,
                            ident,
                            start=False,
                            stop=True,
                            skip_group_check=True,
                        )
                    nc.scalar.activation(
                        et[:, pl * BAND + off : pl * BAND + off + w],
                        pq[:, 0:w],
                        AF.Exp,
                        scale=2.0,
                        accum_out=rowsums[:, rt * 3 + si : rt * 3 + si + 1],
                    )
                    slot = rt * 3 + si
                    if pl == 1:
                        state["pending"].append(
                            (slot, (rt // 2, et, off, w, False))
                        )
                    while state["pending"] and state["pending"][0][0] <= slot - 2:
                        _, args = state["pending"].pop(0)
                        emit_colsum(*args)

            # final pair's remaining colsum groups trail the last exp
            while state["pending"]:
                _, (p, et_pair, off, w, _x) = state["pending"].pop(0)
                emit_colsum(p, et_pair, off, w, not state["pending"])

            # rt7's rowsums column: reduced separately so the first 7 could
            # be reduced early; ship on the idle scalar queue
            nc.vector.tensor_reduce(
                rsum8[:, 7:8],
                rowsums[:, 21:24].rearrange("p (a b) -> p a b", b=3),
                axis=mybir.AxisListType.X,
                op=mybir.AluOpType.add,
            )
            nc.scalar.dma_start(out=rows_d.ap()[:, :], in_=rsum8[:, :])

    _patch_act_table_loads(nc)
    nc.compile()
    return nc


def _act_set_id_with_exp(nc) -> int:
    from concourse.hw_specs import get_activation_tables

    tabs = get_activation_tables(nc.m.arch)
    for i, (name, fns) in enumerate(tabs.items()):
        if AF.Exp in fns:
            return i
    raise RuntimeError("no activation table set with Exp")


def _patch_act_table_loads(nc) -> None:
    # Load the exp table once up front instead of per-switch reloads.
    set_id = _act_set_id_with_exp(nc)

    def _single_act_table_load():
        for blk in nc.main_func.blocks:
            insts = list(blk.instructions)
            for i, ins in enumerate(insts):
                if isinstance(ins, mybir.InstActivation):
                    load = mybir.InstLoadActFuncSet(
                        name=nc.get_next_instruction_name(),
                        act_func_set_id=set_id,
                        ins=[],
                        outs=[],
                    )
                    load.engine = mybir.EngineType.Activation
                    insts.insert(i, load)
                    blk.instructions = insts
                    break

    nc.insert_act_table_loads = _single_act_table_load


def _get_nc() -> bass.Bass:
    if "nc" not in _CACHE:
        _CACHE["nc"] = _build_nc()
    return _CACHE["nc"]


def _masks():
    # lhsT constants for the mask matmuls (rhs = identity):
    # pq[p, t] += mlo[t, p], so mlo[k, m] = NEG iff k <= m masks t <= p
    # (keeps d >= 1); mup[k, m] = NEG iff k > m masks t > p (keeps d <= 4096)
    k = np.arange(128)[:, None]
    m = np.arange(128)[None, :]
    mlo = np.where(k <= m, NEG_MASK, 0.0)
    mup = np.where(k > m, NEG_MASK, 0.0)
    ident = np.eye(128)
    return np.concatenate([mlo, mup, ident], axis=1).astype(NP_F8)


def kernel(emb_i: np.ndarray, emb_j: np.ndarray) -> np.ndarray:
    global LAST_RESULTS
    z = np.concatenate(
        [np.asarray(emb_i, dtype=np.float32), np.asarray(emb_j, dtype=np.float32)],
        axis=0,
    )  # [8192, 256]
    z /= np.maximum(np.sqrt((z * z).sum(axis=1, keepdims=True)), 1e-12)
    z8 = z.astype(NP_F8)           # device values, exact
    z8f = z8.astype(np.float32)
    zt8 = np.ascontiguousarray(z8.T)  # [256, 8192] fp8
    mconst = _masks()

    in_maps = []
    for c in range(N_CORES):
        ztc = zt8 if c == 0 else np.roll(zt8, -c * ROWS, axis=1)
        ztc = ztc[:, :COLS]
        # DoubleRow plane layout: [128, 2*COLS], partition p = dims (p, p+128)
        buf = np.ascontiguousarray(
            np.concatenate([ztc[:128, :], ztc[128:, :]], axis=1)
        )
        in_maps.append({"z8": buf, "mconst": mconst})

    nc = _get_nc()
    LAST_RESULTS = run_bass_kernel_spmd(nc, in_maps, list(range(N_CORES)))

    den = np.zeros(TWO_N, dtype=np.float64)
    band_j = np.arange(BAND)
    for c in range(N_CORES):
        r = LAST_RESULTS.results[c]
        rows = np.asarray(r["rows"], dtype=np.float64)  # [128, 8] (p, rt)
        cols = np.asarray(r["cols"], dtype=np.float64)  # [8, BAND]
        den[c * ROWS : (c + 1) * ROWS] += rows.T.reshape(-1)
        for rt in range(8):
            g = (c * ROWS + rt * 128 + band_j) % TWO_N
            den[g] += cols[rt]

    idx = np.arange(TWO_N)
    pidx = (idx + N) % TWO_N
    # distance-4096 pairs were computed by both endpoints: subtract once,
    # using the same fp8 z the device saw
    pos8 = (z8f[idx] * z8f[pidx]).sum(axis=1, dtype=np.float64)
    den -= np.exp(2.0 * pos8)
    # the loss's positive term uses full-precision z
    pos = (z[idx].astype(np.float64) * z[pidx].astype(np.float64)).sum(axis=1)
    loss = np.mean(np.log(den) - 2.0 * pos)
    return np.array(loss, dtype=np.float32)
